# revision 27
# baseline (speedup 1.0000x reference)
"""Multi-head attention (B=2, N=2048, C=768, H=12) on 8 trn2 cores.

Sharding: core i handles batch b = i//4 and head-group g = i%4 (3 heads each).
Per-core pipeline:
  1. QKV^T projection from host-pre-transposed xT [C, N] (fp32r matmuls):
       qT, kT produced d-major [64, N] per head; v produced n-major [N, 64]
       per head, augmented with a ones column (softmax denominator trick).
  2. Scores computed transposed: S^T[k, q] = kT_h.T-slice @ qT_h, so softmax
       (exp via ScalarE) and attn@V need no on-chip transposes.
  3. attn@V with lhsT = [v | 1]: out rows 0:64 = unnormalized attn_out^T,
       row 64 = softmax denominators.
  4. Normalize (fp32): reciprocal of row 64, PE-broadcast to 64 partitions,
       DVE multiply.
  5. AllGather attn_out^T shards [192, N] -> [768, N] within groups
       [[0,1,2,3],[4,5,6,7]] (same batch).
  6. Output projection column-sharded by w_proj columns (per-core input
       shard), bias folded in as a K=1 matmul. Output is out^T [192, N];
       host concatenates + transposes.

Matmuls run in float32r (1 PE cycle/row vs 4 for fp32); the softmax
normalization chain stays fp32.
"""

import numpy as np

B, N, C, H, HD = 2, 2048, 768, 12, 64
G = 4              # tensor-parallel head groups
HL = H // G        # 3 heads per core
CHL = HL * HD      # 192 local channels
SCALE = HD ** -0.5
NCORES = 8
CT = C // 128      # 6 contraction chunks
NT = N // 128      # 16 n chunks
QW = 1024          # q window width
NWIN = N // QW     # 2 windows
KT = N // 128      # 16 k chunks
FW = 512           # matmul free width (psum bank)

_CACHE = {}


def _build_nc():
    import os
    import concourse.bass as bass
    import concourse.bacc as bacc
    import concourse.tile as tile
    import concourse.mybir as mybir

    F32 = mybir.dt.float32
    F32R = mybir.dt.float32r
    AF = mybir.ActivationFunctionType
    debug = bool(int(os.environ.get("KERNEL_DEBUG", "0")))

    nc = bacc.Bacc(num_devices=NCORES)
    xT_d = nc.declare_dram_parameter("xT", [C, N], F32R, isOutput=False)
    wq_d = nc.declare_dram_parameter("wq", [C, CHL], F32R, isOutput=False)
    wk_d = nc.declare_dram_parameter("wk", [C, CHL], F32R, isOutput=False)
    wv_d = nc.declare_dram_parameter("wv", [C, CHL], F32R, isOutput=False)
    wpz_d = nc.declare_dram_parameter("wpz", [NCORES * CHL, CHL], F32R,
                                      isOutput=False)
    bp_d = nc.declare_dram_parameter("bp", [1, CHL], F32R, isOutput=False)
    out_d = nc.declare_dram_parameter("out", [CHL, N], F32, isOutput=True)
    if debug:
        dbg_sums = nc.declare_dram_parameter("dbg_sums", [HL * NWIN, QW], F32,
                                             isOutput=True)
        dbg_recip = nc.declare_dram_parameter("dbg_recip", [HL * NWIN, QW], F32,
                                              isOutput=True)

    with tile.TileContext(nc) as tc:
        with tc.tile_pool(name="dram", bufs=1, space="DRAM") as dram:
            ag_ins = [dram.tile([CHL, QW], F32R, name=f"ag_in{w}")
                      for w in range(NWIN)]
            ag_outs = [dram.tile([NCORES * CHL, QW], F32R, name=f"ag_out{w}",
                                 addr_space="Shared")
                       for w in range(NWIN)]

            with tc.tile_pool(name="persist", bufs=1) as P:
                # ---- inputs only needed through phase 1 (own pool) ----
                QIN = tc.alloc_tile_pool(name="qkv_in", bufs=1)
                xT_sb = QIN.tile([128, CT, N], F32R)
                for ct in range(CT):
                    nc.sync.dma_start(
                        out=xT_sb[:, ct, :],
                        in_=xT_d[ct * 128:(ct + 1) * 128, :],
                    )
                wq_sb = QIN.tile([128, CT, CHL], F32R)
                wk_sb = QIN.tile([128, CT, CHL], F32R)
                wv_sb = QIN.tile([128, CT, CHL], F32R)
                for wsb, wd in ((wq_sb, wq_d), (wk_sb, wk_d),
                                (wv_sb, wv_d)):
                    for ct in range(CT):
                        nc.sync.dma_start(
                            out=wsb[:, ct, :],
                            in_=wd[ct * 128:(ct + 1) * 128, :],
                        )
                KP = NCORES * CHL // 128   # 12 K-chunks over gathered rows
                wp_sb = P.tile([128, KP, CHL], F32R)
                for kp in range(KP):
                    nc.sync.dma_start(
                        out=wp_sb[:, kp, :],
                        in_=wpz_d[kp * 128:(kp + 1) * 128, :],
                    )
                bp_sb = P.tile([1, CHL], F32R)
                nc.sync.dma_start(out=bp_sb[:], in_=bp_d[:, :])

                ones_sb = P.tile([128, 64], F32)
                nc.vector.memset(ones_sb[:], 1.0)
                ones_q = P.tile([1, FW], F32R)
                nc.vector.memset(ones_q[:].bitcast(F32), 1.0)

                # ---- persistent QKV results ----
                q01_sb = P.tile([128, N], F32R)   # qT heads 0,1
                q2_sb = P.tile([64, N], F32R)     # qT head 2
                k01_sb = P.tile([128, N], F32R)
                k2_sb = P.tile([64, N], F32R)
                # [n, nt, h, 128]: col 0 = ones (sums), 1:64 zero, 64:128 = v
                v_sb = P.tile([128, NT, HL, 2 * HD], F32R)
                nc.vector.memset(v_sb[:, :, :, 0:1].bitcast(F32), 1.0)
                nc.vector.memset(v_sb[:, :, :, 1:HD].bitcast(F32), 0.0)

                # ---- phase 1: QKV projections ----
                with tc.tile_pool(name="qkv_ps", bufs=1, space="PSUM") as QP:
                    for dst, wsb, mlo, mhi in (
                        (q01_sb, wq_sb, 0, 128),
                        (q2_sb, wq_sb, 128, CHL),
                        (k01_sb, wk_sb, 0, 128),
                        (k2_sb, wk_sb, 128, CHL),
                    ):
                        m = mhi - mlo
                        for f in range(N // FW):
                            qk_ps = QP.tile([m, FW], F32, tag="qk", bufs=3,
                                            padded_shape=[128, FW])
                            for ct in range(CT):
                                nc.tensor.matmul(
                                    qk_ps[:],
                                    lhsT=wsb[:, ct, mlo:mhi],
                                    rhs=xT_sb[:, ct, f * FW:(f + 1) * FW],
                                    start=(ct == 0), stop=(ct == CT - 1),
                                )
                            nc.vector.tensor_copy(
                                dst[:, f * FW:(f + 1) * FW], qk_ps[:])
                    for nt in range(NT):
                        v_ps = QP.tile([128, CHL], F32, tag="v", bufs=2)
                        for ct in range(CT):
                            nc.tensor.matmul(
                                v_ps[:],
                                lhsT=xT_sb[:, ct, nt * 128:(nt + 1) * 128],
                                rhs=wv_sb[:, ct, :],
                                start=(ct == 0), stop=(ct == CT - 1),
                            )
                        nc.vector.tensor_copy(
                            v_sb[:, nt, :, HD:2 * HD],
                            v_ps[:].rearrange("p (h d) -> p h d", h=HL))
                QIN.release()

                # ---- phase 2: attention per (head, q-window) ----
                with tc.tile_pool(name="att_ps", bufs=1, space="PSUM") as AT, \
                        tc.tile_pool(name="att_sb", bufs=1) as AS, \
                        tc.tile_pool(name="proj_ps", bufs=1,
                                     space="PSUM") as PP, \
                        tc.tile_pool(name="proj_sb", bufs=1) as PS:
                    for w in range(NWIN):
                        for h in range(HL):
                            qh = (q01_sb[0:64], q01_sb[64:128], q2_sb[0:64])[h]
                            kh = (k01_sb[0:64], k01_sb[64:128], k2_sb[0:64])[h]
                            q0 = w * QW
                            A = AT.tile([128, QW], F32, tag="A", bufs=1)
                            for kc in range(KT):
                                S = AT.tile([128, QW], F32, tag="S", bufs=2)
                                E = AS.tile([128, QW], F32R, tag="E", bufs=4)
                                for j in range(QW // FW):
                                    nc.tensor.matmul(
                                        S[:, j * FW:(j + 1) * FW],
                                        lhsT=kh[:, kc * 128:(kc + 1) * 128],
                                        rhs=qh[:, q0 + j * FW:q0 + (j + 1) * FW],
                                    )
                                nc.scalar.activation(E[:], S[:], AF.Exp,
                                                     scale=SCALE)
                                for j in range(QW // FW):
                                    nc.tensor.matmul(
                                        A[:, j * FW:(j + 1) * FW],
                                        lhsT=v_sb[:, kc, h, :],
                                        rhs=E[:, j * FW:(j + 1) * FW],
                                        start=(kc == 0), stop=(kc == KT - 1),
                                    )
                            # normalize: recip of row 0 (denominators),
                            # gpsimd-broadcast to all partitions, multiply.
                            R = AS.tile([1, QW], F32, tag="R", bufs=2)
                            nc.vector.reciprocal(R[0:1, :], A[0:1, :])
                            bcs = AS.tile([128, QW], F32, tag="bcs", bufs=2)
                            nc.gpsimd.partition_broadcast(bcs[:], R[0:1, :])
                            attn_t = AS.tile([128, QW], F32R, tag="attn",
                                             bufs=3)
                            nc.vector.tensor_mul(attn_t[64:128, :],
                                                 A[64:128, :],
                                                 bcs[64:128, :])
                            nc.sync.dma_start(
                                out=ag_ins[w][h * HD:(h + 1) * HD, :],
                                in_=attn_t[64:128, :],
                            )
                            if debug:
                                dsum = AS.tile([65, QW], F32, tag="dsum",
                                               bufs=2)
                                nc.vector.tensor_copy(dsum[0:1, :],
                                                      A[0:1, :])
                                nc.sync.dma_start(
                                    out=dbg_sums[h * NWIN + w:h * NWIN + w + 1, :],
                                    in_=dsum[0:1, :])
                                nc.sync.dma_start(
                                    out=dbg_recip[h * NWIN + w:h * NWIN + w + 1, :],
                                    in_=R[0:1, :])
                        # per-window 8-core AllGather: window 0's gather
                        # overlaps window 1's attention compute
                        nc.gpsimd.collective_compute(
                            "AllGather",
                            mybir.AluOpType.bypass,
                            replica_groups=[list(range(NCORES))],
                            ins=[ag_ins[w].opt()],
                            outs=[ag_outs[w].opt()],
                        )

                    # ---- phase 4: output projection (out^T [CHL, N]) ----
                    for f in range(N // FW):
                        wf, jf = divmod(f, QW // FW)
                        ao_ts = []
                        for kp in range(KP):
                            ao_t = PS.tile([128, FW], F32R, tag="ao",
                                           bufs=2 * KP)
                            nc.sync.dma_start(
                                out=ao_t[:],
                                in_=ag_outs[wf][kp * 128:(kp + 1) * 128,
                                                jf * FW:(jf + 1) * FW],
                            )
                            ao_ts.append(ao_t)
                        for mlo, mhi in ((0, 128), (128, CHL)):
                            m = mhi - mlo
                            pr_ps = PP.tile([m, FW], F32, tag="pr", bufs=2,
                                            padded_shape=[128, FW])
                            first = True
                            for kp in range(KP):
                                nc.tensor.matmul(
                                    pr_ps[:],
                                    lhsT=wp_sb[:, kp, mlo:mhi],
                                    rhs=ao_ts[kp][:],
                                    start=first, stop=False,
                                )
                                first = False
                            nc.tensor.matmul(
                                pr_ps[:],
                                lhsT=bp_sb[:, mlo:mhi],
                                rhs=ones_q[:],
                                start=False, stop=True,
                            )
                            o_t = PS.tile([m, FW], F32, tag="o", bufs=3,
                                          padded_shape=[128, FW])
                            nc.vector.tensor_copy(o_t[:], pr_ps[:])
                            nc.sync.dma_start(
                                out=out_d[mlo:mhi, f * FW:(f + 1) * FW],
                                in_=o_t[:],
                            )
    nc.finalize()
    return nc


def get_nc():
    if "nc" not in _CACHE:
        _CACHE["nc"] = _build_nc()
    return _CACHE["nc"]


def make_in_maps(x, w_qkv, w_proj, b_proj):
    x = np.asarray(x, dtype=np.float32)
    w_qkv = np.asarray(w_qkv, dtype=np.float32)
    w_proj = np.asarray(w_proj, dtype=np.float32)
    b_proj = np.asarray(b_proj, dtype=np.float32)
    in_maps = []
    for core in range(NCORES):
        b, g = divmod(core, G)
        cs = slice(g * CHL, (g + 1) * CHL)
        im = {
            "xT": np.ascontiguousarray(x[b].T),
            "wq": np.ascontiguousarray(w_qkv[:, 0 * C:1 * C][:, cs]),
            "wk": np.ascontiguousarray(w_qkv[:, 1 * C:2 * C][:, cs]),
            "wv": np.ascontiguousarray(w_qkv[:, 2 * C:3 * C][:, cs]),
            "bp": np.ascontiguousarray(b_proj[cs].reshape(1, CHL)),
        }
        wpz = np.zeros((NCORES * CHL, CHL), np.float32)
        for j in range(NCORES):
            if j // G == b:
                gj = j % G
                wpz[j * CHL:(j + 1) * CHL] = \
                    w_proj[gj * CHL:(gj + 1) * CHL, cs]
        im["wpz"] = wpz
        in_maps.append(im)
    return in_maps


def unshard(results):
    out = np.empty((B, N, C), dtype=np.float32)
    for b in range(B):
        outT = np.concatenate(
            [results[b * G + g]["out"] for g in range(G)], axis=0)
        out[b] = outT.T
    return out


def kernel(x, w_qkv, w_proj, b_proj):
    from concourse.bass_utils import run_bass_kernel_spmd

    nc = get_nc()
    in_maps = make_in_maps(x, w_qkv, w_proj, b_proj)
    res = run_bass_kernel_spmd(nc, in_maps, list(range(NCORES)))
    return unshard(res.results)


# revision 28
# speedup vs baseline: 1.0349x; 1.0349x over previous
"""Multi-head attention (B=2, N=2048, C=768, H=12) on 8 trn2 cores.

Sharding: core i handles batch b = i//4 and head-group g = i%4 (3 heads each).
Per-core pipeline:
  1. QKV^T projection from host-pre-transposed xT [C, N] (fp32r matmuls):
       qT, kT produced d-major [64, N] per head; v produced n-major [N, 64]
       per head, augmented with a ones column (softmax denominator trick).
  2. Scores computed transposed: S^T[k, q] = kT_h.T-slice @ qT_h, so softmax
       (exp via ScalarE) and attn@V need no on-chip transposes.
  3. attn@V with lhsT = [v | 1]: out rows 0:64 = unnormalized attn_out^T,
       row 64 = softmax denominators.
  4. Normalize (fp32): reciprocal of row 64, PE-broadcast to 64 partitions,
       DVE multiply.
  5. AllGather attn_out^T shards [192, N] -> [768, N] within groups
       [[0,1,2,3],[4,5,6,7]] (same batch).
  6. Output projection column-sharded by w_proj columns (per-core input
       shard), bias folded in as a K=1 matmul. Output is out^T [192, N];
       host concatenates + transposes.

Matmuls run in float32r (1 PE cycle/row vs 4 for fp32); the softmax
normalization chain stays fp32.
"""

import numpy as np

B, N, C, H, HD = 2, 2048, 768, 12, 64
G = 4              # tensor-parallel head groups
HL = H // G        # 3 heads per core
CHL = HL * HD      # 192 local channels
SCALE = HD ** -0.5
NCORES = 8
CT = C // 128      # 6 contraction chunks
NT = N // 128      # 16 n chunks
QW = 1024          # q window width
NWIN = N // QW     # 2 windows
KT = N // 128      # 16 k chunks
FW = 512           # matmul free width (psum bank)

_CACHE = {}


def _build_nc():
    import os
    import concourse.bass as bass
    import concourse.bacc as bacc
    import concourse.tile as tile
    import concourse.mybir as mybir

    F32 = mybir.dt.float32
    F32R = mybir.dt.float32r
    AF = mybir.ActivationFunctionType
    debug = bool(int(os.environ.get("KERNEL_DEBUG", "0")))

    nc = bacc.Bacc(num_devices=NCORES)
    xT_d = nc.declare_dram_parameter("xT", [C, N], F32R, isOutput=False)
    wq_d = nc.declare_dram_parameter("wq", [C, CHL], F32R, isOutput=False)
    wk_d = nc.declare_dram_parameter("wk", [C, CHL], F32R, isOutput=False)
    wv_d = nc.declare_dram_parameter("wv", [C, CHL], F32R, isOutput=False)
    wpz_d = nc.declare_dram_parameter("wpz", [NCORES * CHL, CHL], F32R,
                                      isOutput=False)
    bp_d = nc.declare_dram_parameter("bp", [1, CHL], F32R, isOutput=False)
    out_d = nc.declare_dram_parameter("out", [CHL, N], F32, isOutput=True)
    if debug:
        dbg_sums = nc.declare_dram_parameter("dbg_sums", [HL * NWIN, QW], F32,
                                             isOutput=True)
        dbg_recip = nc.declare_dram_parameter("dbg_recip", [HL * NWIN, QW], F32,
                                              isOutput=True)

    with tile.TileContext(nc) as tc:
        with tc.tile_pool(name="dram", bufs=1, space="DRAM") as dram:
            ag_ins = [dram.tile([CHL, QW], F32R, name=f"ag_in{w}")
                      for w in range(NWIN)]
            ag_outs = [dram.tile([NCORES * CHL, QW], F32R, name=f"ag_out{w}",
                                 addr_space="Shared")
                       for w in range(NWIN)]

            with tc.tile_pool(name="persist", bufs=1) as P:
                # ---- inputs only needed through phase 1 (own pool) ----
                QIN = tc.alloc_tile_pool(name="qkv_in", bufs=1)
                xT_sb = QIN.tile([128, CT, N], F32R)
                for ct in range(CT):
                    nc.sync.dma_start(
                        out=xT_sb[:, ct, :],
                        in_=xT_d[ct * 128:(ct + 1) * 128, :],
                    )
                wq_sb = QIN.tile([128, CT, CHL], F32R)
                wk_sb = QIN.tile([128, CT, CHL], F32R)
                wv_sb = QIN.tile([128, CT, CHL], F32R)
                for wsb, wd in ((wq_sb, wq_d), (wk_sb, wk_d),
                                (wv_sb, wv_d)):
                    for ct in range(CT):
                        nc.sync.dma_start(
                            out=wsb[:, ct, :],
                            in_=wd[ct * 128:(ct + 1) * 128, :],
                        )
                KP = NCORES * CHL // 128   # 12 K-chunks over gathered rows
                wp_sb = P.tile([128, KP, CHL], F32R)
                for kp in range(KP):
                    nc.sync.dma_start(
                        out=wp_sb[:, kp, :],
                        in_=wpz_d[kp * 128:(kp + 1) * 128, :],
                    )
                bp_sb = P.tile([1, CHL], F32R)
                nc.sync.dma_start(out=bp_sb[:], in_=bp_d[:, :])

                ones_sb = P.tile([128, 64], F32)
                nc.vector.memset(ones_sb[:], 1.0)
                ones_q = P.tile([1, FW], F32R)
                nc.vector.memset(ones_q[:].bitcast(F32), 1.0)

                # ---- persistent QKV results ----
                q01_sb = P.tile([128, N], F32R)   # qT heads 0,1
                q2_sb = P.tile([64, N], F32R)     # qT head 2
                k01_sb = P.tile([128, N], F32R)
                k2_sb = P.tile([64, N], F32R)
                # [n, nt, h, 128]: col 0 = ones (sums), 1:64 zero, 64:128 = v
                v_sb = P.tile([128, NT, HL, 2 * HD], F32R)
                nc.vector.memset(v_sb[:, :, :, 0:1].bitcast(F32), 1.0)
                nc.vector.memset(v_sb[:, :, :, 1:HD].bitcast(F32), 0.0)

                # ---- phase 1: QKV projections ----
                with tc.tile_pool(name="qkv_ps", bufs=1, space="PSUM") as QP:
                    for dst, wsb, mlo, mhi in (
                        (q01_sb, wq_sb, 0, 128),
                        (q2_sb, wq_sb, 128, CHL),
                        (k01_sb, wk_sb, 0, 128),
                        (k2_sb, wk_sb, 128, CHL),
                    ):
                        m = mhi - mlo
                        for f in range(N // FW):
                            qk_ps = QP.tile([m, FW], F32, tag="qk", bufs=3,
                                            padded_shape=[128, FW])
                            for ct in range(CT):
                                nc.tensor.matmul(
                                    qk_ps[:],
                                    lhsT=wsb[:, ct, mlo:mhi],
                                    rhs=xT_sb[:, ct, f * FW:(f + 1) * FW],
                                    start=(ct == 0), stop=(ct == CT - 1),
                                )
                            nc.vector.tensor_copy(
                                dst[:, f * FW:(f + 1) * FW], qk_ps[:])
                    for nt in range(NT):
                        v_ps = QP.tile([128, CHL], F32, tag="v", bufs=2)
                        for ct in range(CT):
                            nc.tensor.matmul(
                                v_ps[:],
                                lhsT=xT_sb[:, ct, nt * 128:(nt + 1) * 128],
                                rhs=wv_sb[:, ct, :],
                                start=(ct == 0), stop=(ct == CT - 1),
                            )
                        nc.vector.tensor_copy(
                            v_sb[:, nt, :, HD:2 * HD],
                            v_ps[:].rearrange("p (h d) -> p h d", h=HL))
                QIN.release()

                # ---- phase 2: attention per (head, q-window) ----
                with tc.tile_pool(name="att_ps", bufs=1, space="PSUM") as AT, \
                        tc.tile_pool(name="att_sb", bufs=1) as AS:
                    for w in range(NWIN):
                        for h in range(HL):
                            qh = (q01_sb[0:64], q01_sb[64:128], q2_sb[0:64])[h]
                            kh = (k01_sb[0:64], k01_sb[64:128], k2_sb[0:64])[h]
                            q0 = w * QW
                            A = AT.tile([128, QW], F32, tag="A", bufs=1)
                            for kc in range(KT):
                                S = AT.tile([128, QW], F32, tag="S", bufs=3)
                                E = AS.tile([128, QW], F32R, tag="E", bufs=4)
                                for j in range(QW // FW):
                                    nc.tensor.matmul(
                                        S[:, j * FW:(j + 1) * FW],
                                        lhsT=kh[:, kc * 128:(kc + 1) * 128],
                                        rhs=qh[:, q0 + j * FW:q0 + (j + 1) * FW],
                                    )
                                nc.scalar.activation(E[:], S[:], AF.Exp,
                                                     scale=SCALE)
                                for j in range(QW // FW):
                                    nc.tensor.matmul(
                                        A[:, j * FW:(j + 1) * FW],
                                        lhsT=v_sb[:, kc, h, :],
                                        rhs=E[:, j * FW:(j + 1) * FW],
                                        start=(kc == 0), stop=(kc == KT - 1),
                                    )
                            # normalize: recip of row 0 (denominators),
                            # gpsimd-broadcast to all partitions, multiply.
                            R = AS.tile([1, QW], F32, tag="R", bufs=2)
                            nc.vector.reciprocal(R[0:1, :], A[0:1, :])
                            bcs = AS.tile([128, QW], F32, tag="bcs", bufs=2)
                            nc.gpsimd.partition_broadcast(bcs[:], R[0:1, :])
                            attn_t = AS.tile([128, QW], F32R, tag="attn",
                                             bufs=3)
                            nc.vector.tensor_mul(attn_t[64:128, :],
                                                 A[64:128, :],
                                                 bcs[64:128, :])
                            nc.sync.dma_start(
                                out=ag_ins[w][h * HD:(h + 1) * HD, :],
                                in_=attn_t[64:128, :],
                            )
                            if debug:
                                dsum = AS.tile([65, QW], F32, tag="dsum",
                                               bufs=2)
                                nc.vector.tensor_copy(dsum[0:1, :],
                                                      A[0:1, :])
                                nc.sync.dma_start(
                                    out=dbg_sums[h * NWIN + w:h * NWIN + w + 1, :],
                                    in_=dsum[0:1, :])
                                nc.sync.dma_start(
                                    out=dbg_recip[h * NWIN + w:h * NWIN + w + 1, :],
                                    in_=R[0:1, :])
                        # per-window 8-core AllGather: window 0's gather
                        # overlaps window 1's attention compute
                        nc.gpsimd.collective_compute(
                            "AllGather",
                            mybir.AluOpType.bypass,
                            replica_groups=[list(range(NCORES))],
                            ins=[ag_ins[w].opt()],
                            outs=[ag_outs[w].opt()],
                        )

                # ---- phase 4: output projection (out^T [CHL, N]) ----
                with tc.tile_pool(name="proj_ps", bufs=1, space="PSUM") as PP, \
                        tc.tile_pool(name="proj_sb", bufs=1) as PS:
                    for f in range(N // FW):
                        wf, jf = divmod(f, QW // FW)
                        ao_ts = []
                        for kp in range(KP):
                            ao_t = PS.tile([128, FW], F32R, tag="ao",
                                           bufs=2 * KP)
                            nc.sync.dma_start(
                                out=ao_t[:],
                                in_=ag_outs[wf][kp * 128:(kp + 1) * 128,
                                                jf * FW:(jf + 1) * FW],
                            )
                            ao_ts.append(ao_t)
                        for mlo, mhi in ((0, 128), (128, CHL)):
                            m = mhi - mlo
                            pr_ps = PP.tile([m, FW], F32, tag="pr", bufs=4,
                                            padded_shape=[128, FW])
                            first = True
                            for kp in range(KP):
                                nc.tensor.matmul(
                                    pr_ps[:],
                                    lhsT=wp_sb[:, kp, mlo:mhi],
                                    rhs=ao_ts[kp][:],
                                    start=first, stop=False,
                                )
                                first = False
                            nc.tensor.matmul(
                                pr_ps[:],
                                lhsT=bp_sb[:, mlo:mhi],
                                rhs=ones_q[:],
                                start=False, stop=True,
                            )
                            o_t = PS.tile([m, FW], F32, tag="o", bufs=3,
                                          padded_shape=[128, FW])
                            nc.vector.tensor_copy(o_t[:], pr_ps[:])
                            nc.sync.dma_start(
                                out=out_d[mlo:mhi, f * FW:(f + 1) * FW],
                                in_=o_t[:],
                            )
    nc.finalize()
    return nc


def get_nc():
    if "nc" not in _CACHE:
        _CACHE["nc"] = _build_nc()
    return _CACHE["nc"]


def make_in_maps(x, w_qkv, w_proj, b_proj):
    x = np.asarray(x, dtype=np.float32)
    w_qkv = np.asarray(w_qkv, dtype=np.float32)
    w_proj = np.asarray(w_proj, dtype=np.float32)
    b_proj = np.asarray(b_proj, dtype=np.float32)
    in_maps = []
    for core in range(NCORES):
        b, g = divmod(core, G)
        cs = slice(g * CHL, (g + 1) * CHL)
        im = {
            "xT": np.ascontiguousarray(x[b].T),
            "wq": np.ascontiguousarray(w_qkv[:, 0 * C:1 * C][:, cs]),
            "wk": np.ascontiguousarray(w_qkv[:, 1 * C:2 * C][:, cs]),
            "wv": np.ascontiguousarray(w_qkv[:, 2 * C:3 * C][:, cs]),
            "bp": np.ascontiguousarray(b_proj[cs].reshape(1, CHL)),
        }
        wpz = np.zeros((NCORES * CHL, CHL), np.float32)
        for j in range(NCORES):
            if j // G == b:
                gj = j % G
                wpz[j * CHL:(j + 1) * CHL] = \
                    w_proj[gj * CHL:(gj + 1) * CHL, cs]
        im["wpz"] = wpz
        in_maps.append(im)
    return in_maps


def unshard(results):
    out = np.empty((B, N, C), dtype=np.float32)
    for b in range(B):
        outT = np.concatenate(
            [results[b * G + g]["out"] for g in range(G)], axis=0)
        out[b] = outT.T
    return out


def kernel(x, w_qkv, w_proj, b_proj):
    from concourse.bass_utils import run_bass_kernel_spmd

    nc = get_nc()
    in_maps = make_in_maps(x, w_qkv, w_proj, b_proj)
    res = run_bass_kernel_spmd(nc, in_maps, list(range(NCORES)))
    return unshard(res.results)
